# revision 64
# baseline (speedup 1.0000x reference)
"""CrossTransformerLayer on 8 TRN2 NeuronCores.

Sharding: core c -> (batch b = c//2, q-half = c%2). Each core computes its
512 query rows of its batch end-to-end (k/v over the full 1024-token x1
sequence), so no cross-core collectives are needed.

Optimization scheme (v2, fp8 DoubleRow):
  - All large GEMMs (q/k/v projections, output projection, FFN1, FFN2) run
    in fp8e4 with MatmulPerfMode.DoubleRow: contraction pairs ride the AP's
    dim1, halving both instruction count and per-instruction PE time.
  - Power-of-2 scale plumbing keeps every fp8 tensor in range with zero
    extra scaling ops:  y* stored as 8*y (folded into the LN sqrt scale),
    weights hosted at 4x (w2 at 32x), q/k stored bf16 at 32x (exp scale
    0.125/1024), v_aug bf16 at 32x with ones-column 4.0 so oT = 8*o, and
    the residual x kept at 32x (host sends x2h*32, divides output by 32).
  - Softmax bias: the host ships exp(bias) (bf16); the device computes
    e = exp(scores*s) on ACT straight from PSUM (paired kt tiles, 1024
    cols/instr) and multiplies by exp(bias) on DVE (2x bf16 mode) or
    GPSIMD (idle engine), removing the PE identity-preloads and fp32 DVE
    adds of the baseline.
  - x2h loaded once (bf16, 32x) and kept resident for the phase-4 residual.
  - LN transposecopies split DVE/ACT to balance phase-1 engine load.
"""

import sys

sys.path.insert(0, "/opt/trn_rl_repo")

from contextlib import ExitStack

import numpy as np

import concourse.bass as bass
import concourse.tile as tile
from concourse import bacc, mybir
from concourse.masks import make_identity

F32 = mybir.dt.float32
BF16 = mybir.dt.bfloat16
FP8 = mybir.dt.float8e4
DR = mybir.MatmulPerfMode.DoubleRow

B = 4
S = 1024   # full (k) sequence
Sq = 512   # query rows per core
H = 1024
NH = 16
Dh = 64    # head dim
FF = 4096
P = 128
NKT = S // P    # 8 k-token tiles
NFC = H // P    # 8 feature chunks
NTC = Sq // P   # 4 q-token tiles
NFFC = FF // P  # 32 ff chunks
NPR = NFC // 2  # 4 DoubleRow contraction pairs over H
EPS = 1e-5
SY = 8.0        # y (LN output) storage scale
SW = 4.0        # weight storage scale (wq wk wv wo w1)
SX = 32.0       # residual-x / x2h storage scale ( = SY*SW )
SW2 = 32.0      # w2 storage scale
EXPSCALE = 0.125 / (SY * SW * SY * SW)   # psum holds 1024*scores
EXPSHIFT = -3.5  # exp(logit + EXPSHIFT) keeps e in fp8 range; cancels in p
FP8MAX = 240.0
AF = mybir.ActivationFunctionType
OP = mybir.AluOpType

# fraction knob: which exp-bias multiplies go to DVE vs GPSIMD
DVE_BIAS_MOD, DVE_BIAS_LIM = 16, 0
# transposecopies: every Nth stays on DVE, rest on ACT
TCOPY_DVE_MOD = 3


def _pbcast(ap: bass.AP, parts: int) -> bass.AP:
    """[.., N] access pattern -> [parts, .., N] with partition step 0."""
    return bass.AP(
        tensor=ap.tensor,
        offset=ap.offset,
        ap=[[0, parts]] + [list(d) for d in ap.ap],
    )


def _pair0(ap: bass.AP) -> bass.AP:
    """[p, N] access pattern -> [p, 2, N] with dim1 step 0 (DoubleRow pair
    that replays the same data against a hi/res weight pair)."""
    assert len(ap.ap) == 2, ap.ap
    return bass.AP(
        tensor=ap.tensor,
        offset=ap.offset,
        ap=[list(ap.ap[0]), [0, 2], list(ap.ap[1])],
    )


def build_program(ln_affine=True, with_biases=True):
    nc = bacc.Bacc("TRN2", target_bir_lowering=False, debug=False)

    x1_d = nc.dram_tensor("x1", (S, H), BF16, kind="ExternalInput")
    x2h_d = nc.dram_tensor("x2h", (Sq, H), F32, kind="ExternalInput")  # 32x
    ebT_d = nc.dram_tensor("ebT", (NH, S, Sq), FP8, kind="ExternalInput")
    wq_d = nc.dram_tensor("wq", (H, H), FP8, kind="ExternalInput")
    wk_d = nc.dram_tensor("wk", (H, H), FP8, kind="ExternalInput")
    wv_d = nc.dram_tensor("wv", (H, H), FP8, kind="ExternalInput")
    wo_d = nc.dram_tensor("wo", (H, H), FP8, kind="ExternalInput")
    bq_d = nc.dram_tensor("bq_pc", (P, NFC), F32, kind="ExternalInput")
    bk_d = nc.dram_tensor("bk_pc", (P, NFC), F32, kind="ExternalInput")
    bv_d = nc.dram_tensor("bv", (H,), F32, kind="ExternalInput")
    bo_d = nc.dram_tensor("bo", (H,), F32, kind="ExternalInput")
    w1_d = nc.dram_tensor("w1hr", (H, 2, FF), FP8, kind="ExternalInput")
    b1_d = nc.dram_tensor("b1_pc", (P, NFFC), F32, kind="ExternalInput")
    w2_d = nc.dram_tensor("w2hr", (FF, 2, H), FP8, kind="ExternalInput")
    b2_d = nc.dram_tensor("b2", (H,), F32, kind="ExternalInput")
    ln1g_d = nc.dram_tensor("ln1_g", (H,), F32, kind="ExternalInput")
    ln1b_d = nc.dram_tensor("ln1_b", (H,), F32, kind="ExternalInput")
    ln2g_d = nc.dram_tensor("ln2_g", (H,), F32, kind="ExternalInput")
    ln2b_d = nc.dram_tensor("ln2_b", (H,), F32, kind="ExternalInput")
    lnfg_d = nc.dram_tensor("lnf_g", (H,), F32, kind="ExternalInput")
    lnfb_d = nc.dram_tensor("lnf_b", (H,), F32, kind="ExternalInput")
    out_d = nc.dram_tensor("out", (Sq, H), F32, kind="ExternalOutput")

    def _mm_dr(out, lhsT, rhs, **kw):
        nc.tensor.matmul(out, lhsT, rhs, perf_mode=DR, **kw)

    def _layer_norm(pool, y_out, x_in, g_b, b_b, eps_t, stats_on_pool=False):
        """y = 8*(x - mean)/sqrt(var+eps') [* g + 8b] on a [128, H] tile.

        eps_t and the fixed 1/64 sqrt scale encode both the 8x output
        scale and (for 32x-scaled inputs) the input scale: see callers.
        stats_on_pool routes mean/var to GPSIMD (idle in phase 1), via
        explicit sum / sum-of-squares reductions.
        """
        mv = pool.tile([P, 2], F32, tag="ln_mv", name="mv")
        stats = pool.tile([P, 2, 6], F32, tag="ln_stats", name="stats")
        nc.vector.bn_stats(stats[:, 0, :], x_in[:, 0:512])
        nc.vector.bn_stats(stats[:, 1, :], x_in[:, 512:1024])
        nc.vector.bn_aggr(mv, stats)
        std = pool.tile([P, 1], F32, tag="ln_std", name="std")
        nc.scalar.activation(std, mv[:, 1:2], AF.Sqrt, bias=eps_t,
                             scale=1.0 / 64.0)
        rstd = pool.tile([P, 1], F32, tag="ln_rstd", name="rstd")
        nc.vector.reciprocal(rstd, std)
        nc.vector.tensor_scalar(
            y_out, x_in, mv[:, 0:1], rstd, op0=OP.subtract, op1=OP.mult
        )
        if ln_affine:
            nc.vector.tensor_mul(y_out, y_out, g_b)
            nc.vector.tensor_add(y_out, y_out, b_b)

    tcopy_n = 0

    def _tcopy(dst, src, force_dve=False):
        """PSUM->SBUF transposecopy, split DVE/ACT."""
        nonlocal tcopy_n
        tcopy_n += 1
        if force_dve or tcopy_n % TCOPY_DVE_MOD == 0:
            nc.vector.tensor_copy(dst, src)
        else:
            nc.scalar.activation(dst, src, AF.Copy, bias=0.0, scale=1.0)

    def _acopy(dst, src):
        """PSUM->SBUF copy on the Scalar engine."""
        nc.scalar.activation(dst, src, AF.Copy, bias=0.0, scale=1.0)

    with tile.TileContext(nc) as tc, ExitStack() as top:
        persist = top.enter_context(tc.tile_pool(name="persist", bufs=1))
        ident = persist.tile([P, P], BF16, tag="ident")
        make_identity(nc, ident)
        oT = persist.tile([P, NFC, Sq], FP8, tag="oT")      # 8*o [p, fc, q]
        x2_sb = persist.tile([P, NTC, H], F32, tag="x2")    # 32*x2h resident

        xp = top.enter_context(tc.tile_pool(name="xp", bufs=1))
        x_sb = xp.tile([P, NTC, H], BF16, tag="x")          # 32*x [p, tc, f]

        # FFN1 weights: tiles live top-level; loads are emitted interleaved
        # with the attention heads' bias loads (one (g, r) slice per call).
        w1pool = top.enter_context(tc.tile_pool(name="w1l", bufs=8))
        w1cs: list = []
        w1_loaded = [0]

        def _w1_load():
            i = w1_loaded[0]
            if i >= 16:
                return
            g, r = i // 2, i % 2
            if r == 0:
                w1cs.append(
                    w1pool.tile([P, NFC, 2, 512], FP8, tag="w1c",
                                name="w1c")
                )
            nc.sync.dma_start(
                w1cs[g][:, :, r, :],
                w1_d[:, r, g * 512:(g + 1) * 512].rearrange(
                    "(kc p) c -> p kc c", p=P
                ),
            )
            w1_loaded[0] = i + 1
        with (
            tc.tile_pool(name="qkv", bufs=1) as qkvp,           # phases 1-3
            tc.tile_pool(name="bias_s", bufs=3) as bpool,
            tc.tile_pool(name="expp", bufs=3) as epool,
            tc.tile_pool(name="e1p", bufs=4) as e1pool,
            tc.tile_pool(name="rin", bufs=2) as rpool,
        ):
            qT = qkvp.tile([P, NFC, Sq], FP8, tag="qT")         # 32*q
            kT = qkvp.tile([P, NFC, S], FP8, tag="kT")          # 32*k
            bt_pre: list = []
            v_aug = qkvp.tile([P, NKT, NH * 65], FP8, tag="vaug")  # 32*v | 4
            wo_sbs = [
                qkvp.tile([P, NFC, 512], FP8, tag=f"wo{half}",
                          name="wo_sb")
                for half in range(2)
            ]

            # ------------ Phase 1+2: LN, transpose, QKV projections ---------
            with tc.tile_pool(name="y12", bufs=1) as y12:
                y1T = y12.tile([P, NFC, S], FP8, tag="y1T")     # 8*y1
                y2T = y12.tile([P, NFC, Sq], FP8, tag="y2T")    # 8*y2

                with (
                    tc.tile_pool(name="ph1", bufs=5) as ph1,
                    tc.tile_pool(name="ph1w", bufs=4) as ph1w,
                    tc.tile_pool(name="ph1c", bufs=1) as ph1c,
                    tc.tile_pool(
                        name="ph1ps", bufs=5, space=bass.MemorySpace.PSUM
                    ) as ph1ps,
                    tc.tile_pool(name="wload", bufs=1) as wpool,
                    tc.tile_pool(name="vecs", bufs=1) as vecs,
                    tc.tile_pool(
                        name="ph2ps", bufs=3, space=bass.MemorySpace.PSUM
                    ) as ps2,
                ):
                    eps1_t = ph1c.tile([P, 1], F32, tag="eps1")
                    nc.vector.memset(eps1_t, EPS / 64.0)   # true-scale input
                    eps32_t = ph1c.tile([P, 1], F32, tag="eps32")
                    nc.vector.memset(eps32_t, 16.0 * EPS)  # 32x-scaled input
                    ln1g_b = ln1b_b = ln2g_b = ln2b_b = None
                    if ln_affine:
                        ln1g_b = ph1c.tile([P, H], F32, tag="ln1g")
                        ln1b_b = ph1c.tile([P, H], F32, tag="ln1b")
                        ln2g_b = ph1c.tile([P, H], F32, tag="ln2g")
                        ln2b_b = ph1c.tile([P, H], F32, tag="ln2b")
                        nc.gpsimd.dma_start(ln1g_b, _pbcast(ln1g_d[:], P))
                        nc.gpsimd.dma_start(ln1b_b, _pbcast(ln1b_d[:], P))
                        nc.gpsimd.dma_start(ln2g_b, _pbcast(ln2g_d[:], P))
                        nc.gpsimd.dma_start(ln2b_b, _pbcast(ln2b_d[:], P))

                    def _w_full(wd, name, tag):
                        w_sb = wpool.tile([P, NFC, H], FP8, tag=tag, name=name)
                        nc.sync.dma_start(
                            w_sb, wd.rearrange("(kc p) f -> p kc f", p=P)
                        )
                        return w_sb

                    bq_sb = bk_sb = bv_b = None
                    if with_biases:
                        bq_sb = vecs.tile([P, NFC], F32, tag="bq")
                        bk_sb = vecs.tile([P, NFC], F32, tag="bk")
                        bv_b = vecs.tile([P, H], F32, tag="bvb")
                        nc.gpsimd.dma_start(bq_sb, bq_d[:, :])
                        nc.gpsimd.dma_start(bk_sb, bk_d[:, :])
                        nc.gpsimd.dma_start(bv_b, _pbcast(bv_d[:], P))

                    # ones columns of v_aug (slot 64 of each head) = 4.0
                    ones_view = v_aug[:, :, :].rearrange(
                        "p t (h j) -> p t h j", j=65
                    )[:, :, :, 64:65]
                    nc.vector.memset(ones_view, 4.0)

                    def _transpose_to(pspool, yt, yT, t, force_dve=False):
                        """yt [P,H] -> yT[:, :, t*128:(t+1)*128] via grouped
                        4-block PE transposes + one wide copy per half."""
                        for hf in range(2):
                            pt4 = pspool.tile([P, 4, P], BF16, tag="tr",
                                              name="pt")
                            for j in range(4):
                                fc = hf * 4 + j
                                nc.tensor.transpose(
                                    pt4[:, j, :],
                                    yt[:, fc * P:(fc + 1) * P], ident,
                                )
                            _tcopy(
                                yT[:, hf * 4:hf * 4 + 4,
                                   t * P:(t + 1) * P],
                                pt4, force_dve=force_dve,
                            )

                    for t in range(NTC):  # x2h -> y2 -> y2T
                        xt = x2_sb[:, t, :]
                        nc.sync.dma_start(xt, x2h_d[t * P:(t + 1) * P, :])
                        yt = ph1w.tile([P, H], BF16, tag="yt", name="yt")
                        _layer_norm(ph1, yt, xt, ln2g_b, ln2b_b, eps32_t)
                        _transpose_to(ph1ps, yt, y2T, t)

                    wq_sb = _w_full(wq_d[:, :], "wq_sb", tag="wqk")
                    wv_sb = _w_full(wv_d[:, :], "wv_sb", tag="wv")
                    wk_sb = _w_full(wk_d[:, :], "wk_sb", tag="wqk")
                    for half in range(2):
                        nc.sync.dma_start(
                            wo_sbs[half],
                            wo_d[:, half * 512:(half + 1) * 512].rearrange(
                                "(kc p) f -> p kc f", p=P
                            ),
                        )

                    # q^T[fo, :] = sum_j wq[pair j, fo].T @ y2T[pair j]  (DR)
                    for fo in range(NFC):
                        ps = ps2.tile([P, Sq], F32, tag="mm", name="ps")
                        for j in range(NPR):
                            _mm_dr(
                                ps,
                                wq_sb[:, 2 * j:2 * j + 2,
                                      fo * P:(fo + 1) * P],
                                y2T[:, 2 * j:2 * j + 2, :],
                                start=(j == 0),
                                stop=(j == NPR - 1),
                            )
                        if with_biases:
                            nc.vector.tensor_scalar(
                                qT[:, fo, :], ps, bq_sb[:, fo:fo + 1],
                                None, op0=OP.add,
                            )
                        else:
                            _acopy(qT[:, fo, :], ps)

                    # k^T[fo, nt] = DR-sum wk @ y1T: emitted per nt-half as
                    # soon as that half of the x1 ladder lands, so attention
                    # heads can start while the second half is still in LN.
                    def _k_proj(nt):
                        for fo in range(NFC):
                            ps = ps2.tile([P, 512], F32, tag="mm", name="ps")
                            for j in range(NPR):
                                _mm_dr(
                                    ps,
                                    wk_sb[:, 2 * j:2 * j + 2,
                                          fo * P:(fo + 1) * P],
                                    y1T[:, 2 * j:2 * j + 2,
                                        nt * 512:(nt + 1) * 512],
                                    start=(j == 0),
                                    stop=(j == NPR - 1),
                                )
                            if with_biases:
                                nc.vector.tensor_scalar(
                                    kT[:, fo, nt * 512:(nt + 1) * 512],
                                    ps, bk_sb[:, fo:fo + 1], None, op0=OP.add,
                                )
                            else:
                                nc.vector.tensor_copy(
                                    kT[:, fo, nt * 512:(nt + 1) * 512], ps
                                )

                    # x1 -> y1 -> y1T, interleaved with v[t] = y1T[t].T @ wv
                    for t in range(NKT):
                        xt = ph1.tile([P, H], BF16, tag="xt", name="xt")
                        nc.sync.dma_start(xt, x1_d[t * P:(t + 1) * P, :])
                        yt = ph1w.tile([P, H], BF16, tag="yt", name="yt")
                        _layer_norm(ph1, yt, xt, ln1g_b, ln1b_b, eps1_t)
                        # t==7: keep ACT free so the Exp act-table load
                        # overlaps the ladder tail
                        _transpose_to(ph1ps, yt, y1T, t,
                                      force_dve=(t == NKT - 1))
                        for nt in range(2):
                            ps = ps2.tile([P, 512], F32, tag="mm", name="ps")
                            for j in range(NPR):
                                _mm_dr(
                                    ps,
                                    y1T[:, 2 * j:2 * j + 2,
                                        t * P:(t + 1) * P],
                                    wv_sb[:, 2 * j:2 * j + 2,
                                          nt * 512:(nt + 1) * 512],
                                    start=(j == 0),
                                    stop=(j == NPR - 1),
                                )
                            dst = v_aug[
                                :, t, nt * 8 * 65:(nt * 8 + 8) * 65
                            ].rearrange("p (h j) -> p h j", j=65)[:, :, 0:64]
                            if with_biases:
                                nc.vector.tensor_tensor(
                                    out=dst,
                                    in0=ps.rearrange("p (h j) -> p h j", j=64),
                                    in1=bv_b[
                                        :, nt * 512:(nt + 1) * 512
                                    ].rearrange("p (h j) -> p h j", j=64),
                                    op=OP.add,
                                )
                            elif t >= NKT - 2:
                                # ladder tail on DVE: lets ACT drain early so
                                # the Exp act-table load overlaps phase 1
                                nc.vector.tensor_copy(
                                    dst,
                                    ps.rearrange("p (h j) -> p h j", j=64),
                                )
                            else:
                                _acopy(
                                    dst,
                                    ps.rearrange("p (h j) -> p h j", j=64),
                                )
                        if t == 3:
                            _k_proj(0)
                            # prefetch the first heads' exp(bias) tiles so
                            # attention isn't DMA-gated at its early start
                            for h in range(2):
                                bt = bpool.tile([P, NKT, Sq], FP8,
                                                tag="bt", name="bt")
                                nc.sync.dma_start(
                                    bt,
                                    ebT_d[h].rearrange(
                                        "(kt p) q -> p kt q", p=P
                                    ),
                                )
                                bt_pre.append(bt)
                        elif t == NKT - 1:
                            _k_proj(1)

            # ---------------- Phase 3: attention ----------------
            # e = exp(scores*0.125) * exp(bias):  exp on ACT straight from
            # PSUM (paired kt tiles), bias multiply on DVE (2x bf16) or
            # GPSIMD per DVE_BIAS knob. [o^T | rowsum] = [32v|4].T @ e.
            with (
                tc.tile_pool(
                    name="sc_ps", bufs=2, space=bass.MemorySpace.PSUM
                ) as scps,
                tc.tile_pool(
                    name="o_ps", bufs=2, space=bass.MemorySpace.PSUM
                ) as ops,
                tc.tile_pool(name="ph3c", bufs=1) as ph3c,
            ):
                shift_t = ph3c.tile([P, 1], F32, tag="shift")
                nc.vector.memset(shift_t, EXPSHIFT)
                for h in range(NH):
                    hp = (h % 2) * Dh
                    fc = h // 2
                    o_ps = ops.tile([65, Sq], F32, tag="o", name="o_ps")
                    e_t = epool.tile([P, NKT, Sq], FP8, tag="expT",
                                     name="e_t")
                    if h >= 2:
                        # interleave one w1 chunk-load per head so the FFN1
                        # weights stream during attention on SP's idle gaps
                        _w1_load()
                    if h < len(bt_pre):
                        bt = bt_pre[h]
                    else:
                        bt = bpool.tile([P, NKT, Sq], FP8, tag="bt",
                                        name="bt")
                        nc.sync.dma_start(
                            bt, ebT_d[h].rearrange("(kt p) q -> p kt q", p=P)
                        )
                    # kt segments [3,3,2]: wide exp tiles amortize ACT
                    # overhead; the triples' bias-mults go to GPSIMD, the
                    # final pair's to DVE.  o accumulates per kt-pair (DR).
                    op_done = 0
                    for k0, nk in ((0, 3), (3, 3), (6, 2)):
                        sc_ps = scps.tile([P, nk, Sq], F32, tag="sc",
                                          name="sc_ps")
                        for i in range(nk):
                            kt = k0 + i
                            nc.tensor.matmul(
                                sc_ps[:, i, :],
                                kT[hp:hp + Dh, fc, kt * P:(kt + 1) * P],
                                qT[hp:hp + Dh, fc, :],
                                start=True, stop=True,
                            )
                        e1 = e1pool.tile([P, nk, Sq], FP8, tag="e1",
                                         name="e1")
                        nc.scalar.activation(
                            e1, sc_ps, AF.Exp, bias=shift_t, scale=EXPSCALE
                        )
                        eng = nc.gpsimd if nk == 3 else nc.vector
                        eng.tensor_tensor(
                            out=e_t[:, k0:k0 + nk, :],
                            in0=e1,
                            in1=bt[:, k0:k0 + nk, :],
                            op=OP.mult,
                        )
                        while (op_done + 1) * 2 <= k0 + nk:
                            kp = op_done
                            nc.tensor.matmul(
                                o_ps,
                                v_aug[:, 2 * kp:2 * kp + 2,
                                      h * 65:(h + 1) * 65],
                                e_t[:, 2 * kp:2 * kp + 2, :],
                                perf_mode=DR,
                                start=(kp == 0),
                                stop=(kp == NKT // 2 - 1),
                            )
                            op_done += 1
                    if h == NH - 1:
                        while w1_loaded[0] < 16:
                            _w1_load()
                    rinv = rpool.tile([1, Sq], F32, tag="rinv", name="rinv")
                    nc.vector.reciprocal(rinv, o_ps[64:65, :])
                    rb = rpool.tile([Dh, Sq], F32, tag="rb", name="rb")
                    nc.gpsimd.partition_broadcast(rb, rinv[0:1, :])
                    nc.vector.tensor_tensor(
                        out=oT[hp:hp + Dh, fc, :],
                        in0=o_ps[0:64, :], in1=rb,
                        op=OP.mult,
                    )

            # ---------- Phase 4: output projection + residual (DR) ----------
            with (
                tc.tile_pool(name="ph4c", bufs=1) as ph4c,
                tc.tile_pool(
                    name="ph4ps", bufs=3, space=bass.MemorySpace.PSUM
                ) as ps4,
            ):
                bo_b = None
                if with_biases:
                    bo_b = ph4c.tile([P, H], F32, tag="bob")
                    nc.gpsimd.dma_start(bo_b, _pbcast(bo_d[:], P))
                for t in range(NTC):
                    for half in range(2):
                        wo_sb = wo_sbs[half]
                        ps = ps4.tile([P, 512], F32, tag="mm", name="ps")
                        for j in range(NPR):
                            _mm_dr(
                                ps,
                                oT[:, 2 * j:2 * j + 2, t * P:(t + 1) * P],
                                wo_sb[:, 2 * j:2 * j + 2, :],
                                start=(j == 0), stop=(j == NPR - 1),
                            )
                        xs = x_sb[:, t, half * 512:(half + 1) * 512]
                        nc.vector.tensor_tensor(
                            out=xs, in0=ps,
                            in1=x2_sb[:, t, half * 512:(half + 1) * 512],
                            op=OP.add,
                        )
                        if with_biases:
                            nc.vector.tensor_tensor(
                                out=xs, in0=xs,
                                in1=bo_b[:, half * 512:(half + 1) * 512],
                                op=OP.add,
                            )

        # ---------------- Phase 5+6+7: final LN + FFN ----------------
        with tc.tile_pool(name="hT", bufs=1) as hTp:
            hT = hTp.tile([P, NFFC, Sq], FP8, tag="hT")   # gelu(z), true

            with tc.tile_pool(name="y3", bufs=1) as y3p:
                y3T = y3p.tile([P, NFC, Sq], FP8, tag="y3T")   # 8*y3
                with (
                    tc.tile_pool(name="ph5", bufs=4) as ph5,
                    tc.tile_pool(name="ph5w", bufs=2) as ph5w,
                    tc.tile_pool(name="ph5c", bufs=1) as ph5c,
                    tc.tile_pool(
                        name="ph5ps", bufs=4, space=bass.MemorySpace.PSUM
                    ) as ph5ps,
                ):
                    eps_t = ph5c.tile([P, 1], F32, tag="eps")
                    nc.vector.memset(eps_t, 16.0 * EPS)
                    lnfg_b = lnfb_b = None
                    if ln_affine:
                        lnfg_b = ph5c.tile([P, H], F32, tag="lnfg")
                        lnfb_b = ph5c.tile([P, H], F32, tag="lnfb")
                        nc.gpsimd.dma_start(lnfg_b, _pbcast(lnfg_d[:], P))
                        nc.gpsimd.dma_start(lnfb_b, _pbcast(lnfb_d[:], P))
                    for t in range(NTC):
                        yt = ph5w.tile([P, H], BF16, tag="yt", name="yt")
                        _layer_norm(ph5, yt, x_sb[:, t, :], lnfg_b, lnfb_b,
                                    eps_t)
                        for hf in range(2):
                            pt4 = ph5ps.tile([P, 4, P], BF16, tag="tr",
                                             name="pt")
                            for j in range(4):
                                fc = hf * 4 + j
                                nc.tensor.transpose(
                                    pt4[:, j, :],
                                    yt[:, fc * P:(fc + 1) * P], ident,
                                )
                            _tcopy(
                                y3T[:, hf * 4:hf * 4 + 4,
                                    t * P:(t + 1) * P], pt4,
                            )

                # FFN1 + gelu -> hT fully resident in SBUF.  DoubleRow pair
                # carries the (hi, res) split of w1; y3T replays via a
                # step-0 pair AP, so the effective weight is hi+res
                # (~bf16-accurate) at fp8-DR speed.  w1 chunks were loaded
                # during attention (see _w1_load).
                with (
                    tc.tile_pool(name="b1l", bufs=1) as b1pool,
                    tc.tile_pool(
                        name="f1ps", bufs=3, space=bass.MemorySpace.PSUM
                    ) as f1ps,
                ):
                    b1_sb = None
                    if with_biases:
                        b1_sb = b1pool.tile([P, NFFC], F32, tag="b1")
                        nc.gpsimd.dma_start(b1_sb, b1_d[:, :])
                    for g in range(NFFC // 4):
                        w1c = w1cs[g]
                        for i in range(4):
                            ffc = g * 4 + i
                            ps = f1ps.tile([P, Sq], F32, tag="mm", name="ps")
                            for kc in range(NFC):
                                _mm_dr(
                                    ps,
                                    w1c[:, kc, :, i * P:(i + 1) * P],
                                    _pair0(y3T[:, kc, :]),
                                    start=(kc == 0), stop=(kc == NFC - 1),
                                )
                            nc.scalar.activation(
                                hT[:, ffc, :], ps, AF.Gelu,
                                bias=(b1_sb[:, ffc:ffc + 1] if with_biases
                                      else 0.0),
                                scale=1.0 / SX,
                            )

            # FFN2: single pass, full 8-bank PSUM accumulation  (DR)
            with (
                tc.tile_pool(name="w2l", bufs=6) as w2pool,
                tc.tile_pool(name="ph7c", bufs=1) as ph7c,
                tc.tile_pool(name="outp", bufs=2) as outp,
                tc.tile_pool(
                    name="f2ps", bufs=1, space=bass.MemorySpace.PSUM
                ) as f2ps,
            ):
                b2_b = None
                if with_biases:
                    b2_b = ph7c.tile([P, H], F32, tag="b2b")
                    nc.gpsimd.dma_start(b2_b, _pbcast(b2_d[:], P))
                acc = [
                    f2ps.tile([P, H], F32, tag=f"acc{t}", name=f"acc{t}")
                    for t in range(NTC)
                ]
                for g in range(NFFC // 2):
                    w2c = w2pool.tile([P, 2, 2, H], FP8, tag="w2c",
                                      name="w2c")
                    for r in range(2):
                        nc.sync.dma_start(
                            w2c[:, :, r, :],
                            w2_d[g * 256:(g + 1) * 256, r, :].rearrange(
                                "(c p) f -> p c f", p=P
                            ),
                        )
                    for c in range(2):
                        ffc = g * 2 + c
                        for t in range(NTC):
                            for nt in range(2):
                                _mm_dr(
                                    acc[t][:, nt * 512:(nt + 1) * 512],
                                    _pair0(hT[:, ffc, t * P:(t + 1) * P]),
                                    w2c[:, c, :, nt * 512:(nt + 1) * 512],
                                    start=(ffc == 0),
                                    stop=(ffc == NFFC - 1),
                                )
                for t in range(NTC):
                    ot = outp.tile([P, H], F32, tag="ot", name="ot")
                    nc.vector.tensor_tensor(
                        out=ot, in0=acc[t], in1=x_sb[:, t, :], op=OP.add
                    )
                    if with_biases:
                        nc.vector.tensor_tensor(
                            out=ot, in0=ot, in1=b2_b, op=OP.add
                        )
                    nc.sync.dma_start(out_d[t * P:(t + 1) * P, :], ot)

    nc.compile()
    return nc


_CACHE: dict = {}


def _get_program(ln_affine=True, with_biases=True):
    key = (ln_affine, with_biases)
    if key not in _CACHE:
        _CACHE[key] = build_program(
            ln_affine=ln_affine, with_biases=with_biases
        )
    return _CACHE[key]


def _detect_fast_flags(inputs):
    ones = lambda k: bool(np.all(np.asarray(inputs[k]) == 1.0))
    zeros = lambda k: bool(np.all(np.asarray(inputs[k]) == 0.0))
    ln_affine = not (
        ones("ln1_g") and ones("ln2_g") and ones("lnf_g")
        and zeros("ln1_b") and zeros("ln2_b") and zeros("lnf_b")
    )
    with_biases = not (
        zeros("bq") and zeros("bk") and zeros("bv") and zeros("bo")
        and zeros("b1") and zeros("b2")
    )
    return ln_affine, with_biases


def _make_in_maps(inputs: dict) -> list[dict]:
    np_bf = mybir.dt.np(BF16)
    np_f8 = mybir.dt.np(FP8)
    f32 = lambda a: np.ascontiguousarray(np.asarray(a, dtype=np.float32))
    bf16 = lambda a: np.ascontiguousarray(
        np.asarray(a, dtype=np.float32).astype(np_bf)
    )

    def fp8(a, scale):
        a = np.asarray(a, dtype=np.float32) * scale
        return np.ascontiguousarray(
            np.clip(a, -FP8MAX, FP8MAX).astype(np_f8)
        )

    def fp8_hr(a, scale):
        """[K, N] -> [K, 2, N] fp8 (hi, residual) pair at `scale`."""
        a = np.asarray(a, dtype=np.float32) * scale
        hi = np.clip(a, -FP8MAX, FP8MAX).astype(np_f8)
        res = np.clip(a - hi.astype(np.float32), -FP8MAX, FP8MAX
                      ).astype(np_f8)
        return np.ascontiguousarray(np.stack([hi, res], axis=1))

    x1 = np.asarray(inputs["x1"], dtype=np.float32)
    x2 = np.asarray(inputs["x2"], dtype=np.float32)
    attn_bias = np.asarray(inputs["attn_bias"], dtype=np.float32)
    shared = {
        "wq": fp8(inputs["wq"], SW),
        "wk": fp8(inputs["wk"], SW),
        "wv": fp8(inputs["wv"], SW),
        "wo": fp8(inputs["wo"], SW),
        "bq_pc": f32(np.asarray(inputs["bq"]).reshape(NFC, P).T * SX),
        "bk_pc": f32(np.asarray(inputs["bk"]).reshape(NFC, P).T * SX),
        "bv": f32(np.asarray(inputs["bv"]) * SX),
        "bo": f32(np.asarray(inputs["bo"]) * SX),
        "w1hr": fp8_hr(inputs["w1"], SW),
        "b1_pc": f32(np.asarray(inputs["b1"]).reshape(NFFC, P).T),
        "w2hr": fp8_hr(inputs["w2"], SW2),
        "b2": f32(np.asarray(inputs["b2"]) * SX),
        "ln1_g": f32(inputs["ln1_g"]),
        "ln1_b": f32(np.asarray(inputs["ln1_b"]) * SY),
        "ln2_g": f32(inputs["ln2_g"]),
        "ln2_b": f32(np.asarray(inputs["ln2_b"]) * SY),
        "lnf_g": f32(inputs["lnf_g"]),
        "lnf_b": f32(np.asarray(inputs["lnf_b"]) * SY),
    }
    in_maps = []
    for c in range(8):
        b, half = c // 2, c % 2
        q0 = half * Sq
        eb = np.exp(attn_bias[b, :, q0:q0 + Sq, :]).transpose(0, 2, 1)
        eb = np.clip(eb, 0.0, FP8MAX).astype(np_f8)
        in_maps.append(
            {
                "x1": bf16(x1[b]),
                "x2h": f32(x2[b, q0:q0 + Sq] * SX),
                "ebT": np.ascontiguousarray(eb),
                **shared,
            }
        )
    return in_maps


def _assemble(results: list[dict]) -> np.ndarray:
    out = np.empty((B, S, H), np.float32)
    for c in range(8):
        b, half = c // 2, c % 2
        out[b, half * Sq:(half + 1) * Sq] = results[c]["out"] * (1.0 / SX)
    return out


def run(inputs: dict, **run_kwargs):
    from concourse.bass_utils import run_bass_kernel_spmd

    ln_affine, with_biases = _detect_fast_flags(inputs)
    nc = _get_program(ln_affine=ln_affine, with_biases=with_biases)
    in_maps = _make_in_maps(inputs)
    res = run_bass_kernel_spmd(nc, in_maps, core_ids=list(range(8)),
                               **run_kwargs)
    return _assemble(res.results), res


def kernel(**inputs) -> np.ndarray:
    out, _ = run(inputs)
    return out
